# revision 6
# baseline (speedup 1.0000x reference)
"""GCN layer on 8 Trainium2 NeuronCores.

Computes relu(D^-1/2 (A+I) D^-1/2 X W + b) for N=8192, d=256.

Sharding: row-shard adj over N across the 8 cores (1024 rows each); x, W, b
replicated. Each core's shard is laid out column-major (adj[rows].T) so the
contraction dim (adj columns j) lands on SBUF partitions, which the PE array
requires for A @ X. On device, each core:
  1. streams its 32MB fp32 shard once with an inline fp32->bf16 cast-DMA into
     a persistent SBUF cache, while the tensor engine accumulates row sums
     (matmul against a ones vector),
  2. AllGathers the 8 local degree vectors (4KB each),
  3. computes y = D^-1/2 x, U^T = (A y)^T from the SBUF cache (+I via
     identity-matmul of its own y rows), scales by D^-1/2 on the output side,
     applies W, bias and ReLU, and writes its output block transposed.
The host transposes/concatenates the 8 output blocks.
"""

import numpy as np

N = 8192
D = 256
NCORES = 8
R = N // NCORES  # rows per core = 1024
KT = N // 128  # 64 j-tiles
TS = R // 128  # 8 own-row tiles

_CACHE = {}


def _build_nc():
    import concourse.bacc as bacc
    import concourse.tile as tile
    import concourse.mybir as mybir

    f32 = mybir.dt.float32
    bf16 = mybir.dt.bfloat16

    nc = bacc.Bacc("TRN2", target_bir_lowering=False, debug=False,
                   num_devices=NCORES)

    adjT = nc.dram_tensor("adjT", [N, R], f32, kind="ExternalInput")
    xin = nc.dram_tensor("x", [N, D], f32, kind="ExternalInput")
    xown = nc.dram_tensor("xown", [R, D], f32, kind="ExternalInput")
    Win = nc.dram_tensor("W", [D, D], f32, kind="ExternalInput")
    bin_ = nc.dram_tensor("b", [D], f32, kind="ExternalInput")
    eye = nc.dram_tensor("eye", [128, 128], bf16, kind="ExternalInput")
    outT = nc.dram_tensor("outT", [D, R], f32, kind="ExternalOutput")

    with tile.TileContext(nc) as tc:
        from contextlib import ExitStack

        with ExitStack() as ctx:
            pp = ctx.enter_context(tc.tile_pool(name="persist", bufs=1))
            dp = ctx.enter_context(tc.tile_pool(name="dram", bufs=1, space="DRAM"))

            # ---- persistent SBUF tensors ----
            adjTb = pp.tile([128, KT * R], bf16)   # 128KB/partition cache
            xb = pp.tile([128, KT * D], bf16)      # x, partition = j%128
            xob = pp.tile([128, TS * D], bf16)     # own x rows
            Wb = pp.tile([128, 2 * D], bf16)       # W, partition = n%128
            bsb = pp.tile([128, 2], f32)           # bias, partition = m%128
            eyeb = pp.tile([128, 128], bf16)
            ones_bf = pp.tile([128, 1], bf16)
            ones_row = pp.tile([1, 128], f32)
            deg_s = pp.tile([1, R], f32)           # local degree (row sums + 1)
            disl = pp.tile([1, R], f32)            # local D^-1/2
            dis_pp = pp.tile([128, KT], f32)       # full D^-1/2, partition = j%128
            diso_pp = pp.tile([128, TS], f32)      # own D^-1/2, partition = i%128
            disrep = pp.tile([128, R], f32)        # own D^-1/2 on free dim, bcast
            y2 = [pp.tile([128, R], bf16, name=f"y2_{i}") for i in range(2)]
            outsb = [pp.tile([128, R], f32, name=f"outsb_{i}") for i in range(2)]

            degl_d = dp.tile([R], f32)
            dega_d = dp.tile([N], f32)

            nc.any.memset(ones_bf[:], 1.0)
            nc.any.memset(ones_row[:], 1.0)

            # ---- phase 1: stream adjT (cast to bf16) + row sums on PE ----
            GC = 8  # j-tiles per DMA chunk (4MB fp32 each)
            for g in range(KT // GC):
                src = adjT.ap()[g * GC * 128:(g + 1) * GC * 128, :].rearrange(
                    "(k p) i -> p k i", p=128)
                dst = adjTb[:, g * GC * R:(g + 1) * GC * R].rearrange(
                    "p (k i) -> p k i", i=R)
                nc.gpsimd.dma_start(out=dst, in_=src)

            with tc.tile_pool(name="psdeg", bufs=1, space="PSUM") as pdeg:
                d0 = pdeg.tile([1, 512], f32)
                d1 = pdeg.tile([1, 512], f32)
                for k in range(KT):
                    nc.tensor.matmul(
                        d0[:, :], ones_bf[:, :],
                        adjTb[:, k * R:k * R + 512],
                        start=(k == 0), stop=(k == KT - 1))
                    nc.tensor.matmul(
                        d1[:, :], ones_bf[:, :],
                        adjTb[:, k * R + 512:(k + 1) * R],
                        start=(k == 0), stop=(k == KT - 1))
                # deg = rowsum + 1 (the +I term)
                nc.vector.tensor_scalar_add(deg_s[:, 0:512], d0[:, :], 1.0)
                nc.vector.tensor_scalar_add(deg_s[:, 512:1024], d1[:, :], 1.0)

            # local dis = 1/sqrt(deg), for the broadcast row
            nc.vector.reciprocal_approx_fast(disl[:, :], deg_s[:, :])
            nc.scalar.activation(disl[:, :], disl[:, :],
                                 mybir.ActivationFunctionType.Sqrt)

            # ---- small input loads (queued behind adjT on purpose) ----
            nc.gpsimd.dma_start(
                out=xb[:, :].rearrange("p (k n) -> p k n", n=D),
                in_=xin.ap().rearrange("(k p) n -> p k n", p=128))
            nc.gpsimd.dma_start(
                out=xob[:, :].rearrange("p (t n) -> p t n", n=D),
                in_=xown.ap().rearrange("(t p) n -> p t n", p=128))
            nc.gpsimd.dma_start(
                out=Wb[:, :].rearrange("p (k m) -> p k m", m=D),
                in_=Win.ap().rearrange("(k p) m -> p k m", p=128))
            nc.sync.dma_start(
                out=bsb[:, :], in_=bin_.ap().rearrange("(h p) -> p h", p=128))
            nc.sync.dma_start(out=eyeb[:, :], in_=eye.ap())

            # ---- phase 2: AllGather degrees ----
            nc.sync.dma_start(out=degl_d[:], in_=deg_s[0:1, :])
            nc.gpsimd.collective_compute(
                "AllGather",
                mybir.AluOpType.bypass,
                replica_groups=[list(range(NCORES))],
                ins=[degl_d.opt()],
                outs=[dega_d.opt()],
            )

            # full dis, partition-aligned with x chunks
            nc.sync.dma_start(
                out=dis_pp[:, :], in_=dega_d.opt().rearrange("(k p) -> p k", p=128))
            nc.vector.reciprocal_approx_fast(dis_pp[:, :], dis_pp[:, :])
            nc.scalar.activation(dis_pp[:, :], dis_pp[:, :],
                                 mybir.ActivationFunctionType.Sqrt)
            # own dis, partition-aligned with own rows
            nc.sync.dma_start(
                out=diso_pp[:, :], in_=degl_d.opt().rearrange("(t p) -> p t", p=128))
            nc.vector.reciprocal_approx_fast(diso_pp[:, :], diso_pp[:, :])
            nc.scalar.activation(diso_pp[:, :], diso_pp[:, :],
                                 mybir.ActivationFunctionType.Sqrt)

            # y = dis * x (in place on xb), same for own rows
            nc.vector.tensor_mul(
                xb[:, :].rearrange("p (k n) -> p k n", n=D),
                xb[:, :].rearrange("p (k n) -> p k n", n=D),
                dis_pp[:, :].unsqueeze(2).broadcast_to([128, KT, D]))
            nc.vector.tensor_mul(
                xob[:, :].rearrange("p (t n) -> p t n", n=D),
                xob[:, :].rearrange("p (t n) -> p t n", n=D),
                diso_pp[:, :].unsqueeze(2).broadcast_to([128, TS, D]))

            # broadcast own dis along free dim: disrep[p, i] = disl[0, i]
            with tc.tile_pool(name="psbc", bufs=1, space="PSUM") as pbc:
                bc = pbc.tile([128, R], f32)
                nc.tensor.matmul(bc[:, 0:512], ones_row[:, :], disl[:, 0:512],
                                 start=True, stop=True)
                nc.tensor.matmul(bc[:, 512:1024], ones_row[:, :],
                                 disl[:, 512:1024], start=True, stop=True)
                nc.vector.tensor_copy(disrep[:, :], bc[:, :])

            # ---- phase 3: U^T = (A y)^T, accumulated over j-tiles ----
            with (
                tc.tile_pool(name="psu", bufs=1, space="PSUM") as pu,
                tc.tile_pool(name="pso", bufs=1, space="PSUM") as po,
            ):
                u = [pu.tile([128, R], f32, name=f"u_{i}") for i in range(2)]
                for k in range(KT):
                    for h in range(2):
                        for s in range(2):
                            nc.tensor.matmul(
                                u[h][:, s * 512:(s + 1) * 512],
                                xb[:, k * D + h * 128:k * D + (h + 1) * 128],
                                adjTb[:, k * R + s * 512:k * R + (s + 1) * 512],
                                start=(k == 0), stop=False,
                                skip_group_check=True)
                # +I: U^T[n, own block t] += y_own[t]^T
                for t in range(TS):
                    for h in range(2):
                        nc.tensor.matmul(
                            u[h][:, t * 128:(t + 1) * 128],
                            xob[:, t * D + h * 128:t * D + (h + 1) * 128],
                            eyeb[:, :],
                            start=False, stop=(t == TS - 1),
                            skip_group_check=True)

                # ---- phase 4: scale columns by own dis, cast to bf16 ----
                for h in range(2):
                    nc.vector.tensor_mul(y2[h][:, :], u[h][:, :], disrep[:, :])

                # ---- phase 5: out^T = W^T @ (scaled U^T) ----
                o = [po.tile([128, R], f32, name=f"o_{i}") for i in range(2)]
                for mh in range(2):
                    for nk in range(2):
                        for s in range(2):
                            nc.tensor.matmul(
                                o[mh][:, s * 512:(s + 1) * 512],
                                Wb[:, nk * D + mh * 128:nk * D + (mh + 1) * 128],
                                y2[nk][:, s * 512:(s + 1) * 512],
                                start=(nk == 0), stop=(nk == 1),
                                skip_group_check=True)

                # ---- phase 6: relu(out^T + b), write transposed output ----
                for mh in range(2):
                    nc.scalar.activation(
                        outsb[mh][:, :], o[mh][:, :],
                        mybir.ActivationFunctionType.Relu,
                        bias=bsb[:, mh:mh + 1], scale=1.0)
                    nc.sync.dma_start(
                        out=outT.ap()[mh * 128:(mh + 1) * 128, :],
                        in_=outsb[mh][:, :])

    nc.compile()
    return nc


def _get_nc():
    if "nc" not in _CACHE:
        _CACHE["nc"] = _build_nc()
    return _CACHE["nc"]


def kernel(x, adj, W, b):
    import ml_dtypes
    from concourse.bass_utils import run_bass_kernel_spmd

    x = np.ascontiguousarray(np.asarray(x, dtype=np.float32))
    adj = np.asarray(adj, dtype=np.float32)
    W = np.ascontiguousarray(np.asarray(W, dtype=np.float32))
    b = np.ascontiguousarray(np.asarray(b, dtype=np.float32))

    nc = _get_nc()

    eye_np = np.eye(128, dtype=ml_dtypes.bfloat16)
    in_maps = []
    for c in range(NCORES):
        rows = slice(c * R, (c + 1) * R)
        in_maps.append({
            "adjT": np.ascontiguousarray(adj[rows, :].T),
            "x": x,
            "xown": np.ascontiguousarray(x[rows, :]),
            "W": W,
            "b": b,
            "eye": eye_np,
        })

    res = run_bass_kernel_spmd(nc, in_maps, core_ids=list(range(NCORES)))
    out = np.concatenate(
        [np.asarray(res.results[c]["outT"]).T for c in range(NCORES)], axis=0)
    return np.ascontiguousarray(out, dtype=np.float32)


if __name__ == "__main__":
    rng = np.random.default_rng(0)
    x = rng.standard_normal((N, D)).astype(np.float32)
    adj = rng.random((N, N)).astype(np.float32)
    W = rng.standard_normal((D, D)).astype(np.float32) * 0.06
    b = rng.standard_normal((D,)).astype(np.float32) * 0.06
    out = kernel(x=x, adj=adj, W=W, b=b)
    print(out.shape, out.dtype)


# revision 7
# speedup vs baseline: 1.2989x; 1.2989x over previous
"""GCN layer on 8 Trainium2 NeuronCores.

Computes relu(D^-1/2 (A+I) D^-1/2 X W + b) for N=8192, d=256.

Sharding: row-shard adj over N across the 8 cores (1024 rows each); x, W, b
replicated. Each core's shard is laid out column-major (adj[rows].T) and in
bf16 (the tensor-engine compute precision) so the contraction dim (adj
columns j) lands on SBUF partitions and the stream is half the bytes. On
device, each core:
  1. streams its 16MB shard once over HWDGE into a persistent SBUF cache,
     while the tensor engine accumulates row sums (matmul against ones),
  2. AllGathers the 8 local degree vectors (4KB each),
  3. computes y = D^-1/2 x chunk-by-chunk interleaved with the U^T = (A y)^T
     matmuls from the SBUF cache (+I via identity-matmul of its own y rows),
     scales by D^-1/2 on the output side, applies W, bias and ReLU, and
     writes its output block transposed.
The host transposes/concatenates the 8 output blocks.
"""

import numpy as np

N = 8192
D = 256
NCORES = 8
R = N // NCORES  # rows per core = 1024
KT = N // 128  # 64 j-tiles
TS = R // 128  # 8 own-row tiles

_CACHE = {}


def _build_nc():
    import concourse.bacc as bacc
    import concourse.tile as tile
    import concourse.mybir as mybir

    f32 = mybir.dt.float32
    bf16 = mybir.dt.bfloat16

    nc = bacc.Bacc("TRN2", target_bir_lowering=False, debug=False,
                   num_devices=NCORES)

    adjT = nc.dram_tensor("adjT", [N, R], bf16, kind="ExternalInput")
    xin = nc.dram_tensor("x", [N, D], bf16, kind="ExternalInput")
    xown = nc.dram_tensor("xown", [R, D], bf16, kind="ExternalInput")
    Win = nc.dram_tensor("W", [D, D], bf16, kind="ExternalInput")
    bin_ = nc.dram_tensor("b", [D], f32, kind="ExternalInput")
    eyeb = nc.dram_tensor("eye", [128, 128], bf16, kind="ExternalInput")
    eyef = nc.dram_tensor("eyef", [128, 128], f32, kind="ExternalInput")
    outT = nc.dram_tensor("outT", [D, R], f32, kind="ExternalOutput")

    with tile.TileContext(nc) as tc:
        from contextlib import ExitStack

        with ExitStack() as ctx:
            pp = ctx.enter_context(tc.tile_pool(name="persist", bufs=1))
            dp = ctx.enter_context(tc.tile_pool(name="dram", bufs=1, space="DRAM"))

            # ---- persistent SBUF tensors ----
            adjTb = pp.tile([128, KT * R], bf16)   # 128KB/partition cache
            xb = pp.tile([128, KT * D], bf16)      # x, partition = j%128
            xob = pp.tile([128, TS * D], bf16)     # own x rows
            Wb = pp.tile([128, 2 * D], bf16)       # W, partition = n%128
            bsb = pp.tile([128, 2], f32)           # bias, partition = m%128
            eye_s = pp.tile([128, 128], bf16)
            eyef_s = pp.tile([128, 128], f32)
            ones_bf = pp.tile([128, 1], bf16)
            ones_row = pp.tile([1, 128], f32)
            deg_s = pp.tile([1, R], f32)           # local degree (row sums + 1)
            disl = pp.tile([1, R], f32)            # local D^-1/2
            degn = pp.tile([64, 128], f32)         # gathered degrees, natural
            degln = pp.tile([8, 128], f32)         # local degrees, natural
            dis_pp = pp.tile([128, KT], f32)       # full D^-1/2, partition = j%128
            diso_pp = pp.tile([128, TS], f32)      # own D^-1/2, partition = i%128
            disrep = pp.tile([128, R], f32)        # own D^-1/2 on free dim, bcast
            y2 = [pp.tile([128, R], bf16, name=f"y2_{i}") for i in range(2)]
            outsb = [pp.tile([128, R], f32, name=f"outsb_{i}") for i in range(2)]

            degl_d = dp.tile([R], f32)
            dega_d = dp.tile([N], f32)

            nc.any.memset(ones_bf[:], 1.0)
            nc.any.memset(ones_row[:], 1.0)

            # ---- small loads first (sync HWDGE queue), then the big stream
            nc.sync.dma_start(
                out=xob[:, :].rearrange("p (t n) -> p t n", n=D),
                in_=xown.ap().rearrange("(t p) n -> p t n", p=128))
            nc.sync.dma_start(
                out=Wb[:, :].rearrange("p (k m) -> p k m", m=D),
                in_=Win.ap().rearrange("(k p) m -> p k m", p=128))
            nc.sync.dma_start(
                out=bsb[:, :], in_=bin_.ap().rearrange("(h p) -> p h", p=128))
            nc.sync.dma_start(out=eye_s[:, :], in_=eyeb.ap())
            nc.sync.dma_start(out=eyef_s[:, :], in_=eyef.ap())

            # ---- phase 1: stream adjT + row sums on PE ----
            GC = 8  # j-tiles per DMA chunk (2MB bf16 each)
            for g in range(KT // GC):
                src = adjT.ap()[g * GC * 128:(g + 1) * GC * 128, :].rearrange(
                    "(k p) i -> p k i", p=128)
                dst = adjTb[:, g * GC * R:(g + 1) * GC * R].rearrange(
                    "p (k i) -> p k i", i=R)
                nc.sync.dma_start(out=dst, in_=src)
            # x streamed after adjT; needed only once degrees are gathered
            nc.sync.dma_start(
                out=xb[:, :].rearrange("p (k n) -> p k n", n=D),
                in_=xin.ap().rearrange("(k p) n -> p k n", p=128))

            with tc.tile_pool(name="psdeg", bufs=1, space="PSUM") as pdeg:
                d0 = pdeg.tile([1, 512], f32)
                d1 = pdeg.tile([1, 512], f32)
                for k in range(KT):
                    nc.tensor.matmul(
                        d0[:, :], ones_bf[:, :],
                        adjTb[:, k * R:k * R + 512],
                        start=(k == 0), stop=(k == KT - 1))
                    nc.tensor.matmul(
                        d1[:, :], ones_bf[:, :],
                        adjTb[:, k * R + 512:(k + 1) * R],
                        start=(k == 0), stop=(k == KT - 1))
                # deg = rowsum + 1 (the +I term)
                nc.vector.tensor_scalar_add(deg_s[:, 0:512], d0[:, :], 1.0)
                nc.vector.tensor_scalar_add(deg_s[:, 512:1024], d1[:, :], 1.0)

            # local dis = 1/sqrt(deg), for the free-dim broadcast row
            nc.vector.reciprocal_approx_fast(disl[:, :], deg_s[:, :])
            nc.scalar.activation(disl[:, :], disl[:, :],
                                 mybir.ActivationFunctionType.Sqrt)

            # ---- phase 2: AllGather degrees (scalar HWDGE + gpsimd CC) ----
            nc.scalar.dma_start(out=degl_d[:], in_=deg_s[0:1, :])
            nc.gpsimd.collective_compute(
                "AllGather",
                mybir.AluOpType.bypass,
                replica_groups=[list(range(NCORES))],
                ins=[degl_d.opt()],
                outs=[dega_d.opt()],
            )

            # degrees -> per-partition D^-1/2 via natural load + PE transpose
            nc.scalar.dma_start(
                out=degn[:, :], in_=dega_d.opt().rearrange("(c f) -> c f", f=128))
            nc.scalar.dma_start(
                out=degln[:, :], in_=degl_d.opt().rearrange("(c f) -> c f", f=128))
            with tc.tile_pool(name="pst", bufs=1, space="PSUM") as pst:
                tdeg = pst.tile([128, 64], f32)
                tdegl = pst.tile([128, 8], f32)
                nc.tensor.transpose(tdeg[:, :], degn[:, :], eyef_s[0:64, 0:64])
                nc.tensor.transpose(tdegl[:, :], degln[:, :], eyef_s[0:8, 0:8])
                nc.vector.reciprocal_approx_fast(dis_pp[:, :], tdeg[:, :])
                nc.vector.reciprocal_approx_fast(diso_pp[:, :], tdegl[:, :])
            nc.scalar.activation(dis_pp[:, :], dis_pp[:, :],
                                 mybir.ActivationFunctionType.Sqrt)
            nc.scalar.activation(diso_pp[:, :], diso_pp[:, :],
                                 mybir.ActivationFunctionType.Sqrt)

            # broadcast own dis along free dim: disrep[p, i] = disl[0, i]
            with tc.tile_pool(name="psbc", bufs=1, space="PSUM") as pbc:
                bc = pbc.tile([128, R], f32)
                nc.tensor.matmul(bc[:, 0:512], ones_row[:, :], disl[:, 0:512],
                                 start=True, stop=True)
                nc.tensor.matmul(bc[:, 512:1024], ones_row[:, :],
                                 disl[:, 512:1024], start=True, stop=True)
                nc.vector.tensor_copy(disrep[:, :], bc[:, :])

            # ---- phase 3: y = dis*x and U^T = (A y)^T, interleaved per tile
            with (
                tc.tile_pool(name="psu", bufs=1, space="PSUM") as pu,
                tc.tile_pool(name="pso", bufs=1, space="PSUM") as po,
            ):
                u = [pu.tile([128, R], f32, name=f"u_{i}") for i in range(2)]
                for k in range(KT):
                    nc.vector.tensor_scalar_mul(
                        xb[:, k * D:(k + 1) * D],
                        xb[:, k * D:(k + 1) * D],
                        dis_pp[:, k:k + 1])
                    for h in range(2):
                        for s in range(2):
                            nc.tensor.matmul(
                                u[h][:, s * 512:(s + 1) * 512],
                                xb[:, k * D + h * 128:k * D + (h + 1) * 128],
                                adjTb[:, k * R + s * 512:k * R + (s + 1) * 512],
                                start=(k == 0), stop=False,
                                skip_group_check=True)
                # +I: U^T[n, own block t] += y_own[t]^T
                for t in range(TS):
                    nc.vector.tensor_scalar_mul(
                        xob[:, t * D:(t + 1) * D],
                        xob[:, t * D:(t + 1) * D],
                        diso_pp[:, t:t + 1])
                    for h in range(2):
                        nc.tensor.matmul(
                            u[h][:, t * 128:(t + 1) * 128],
                            xob[:, t * D + h * 128:t * D + (h + 1) * 128],
                            eye_s[:, :],
                            start=False, stop=(t == TS - 1),
                            skip_group_check=True)

                # ---- phase 4: scale columns by own dis, cast to bf16 ----
                for h in range(2):
                    nc.vector.tensor_mul(y2[h][:, :], u[h][:, :], disrep[:, :])

                # ---- phase 5: out^T = W^T @ (scaled U^T) ----
                o = [po.tile([128, R], f32, name=f"o_{i}") for i in range(2)]
                for mh in range(2):
                    for nk in range(2):
                        for s in range(2):
                            nc.tensor.matmul(
                                o[mh][:, s * 512:(s + 1) * 512],
                                Wb[:, nk * D + mh * 128:nk * D + (mh + 1) * 128],
                                y2[nk][:, s * 512:(s + 1) * 512],
                                start=(nk == 0), stop=(nk == 1),
                                skip_group_check=True)

                # ---- phase 6: relu(out^T + b), write transposed output ----
                for mh in range(2):
                    nc.scalar.activation(
                        outsb[mh][:, :], o[mh][:, :],
                        mybir.ActivationFunctionType.Relu,
                        bias=bsb[:, mh:mh + 1], scale=1.0)
                    nc.sync.dma_start(
                        out=outT.ap()[mh * 128:(mh + 1) * 128, :],
                        in_=outsb[mh][:, :])

    nc.compile()
    return nc


def _get_nc():
    if "nc" not in _CACHE:
        _CACHE["nc"] = _build_nc()
    return _CACHE["nc"]


def kernel(x, adj, W, b):
    import ml_dtypes
    from concourse.bass_utils import run_bass_kernel_spmd

    bf = ml_dtypes.bfloat16
    x = np.asarray(x, dtype=np.float32)
    adj = np.asarray(adj, dtype=np.float32)
    W = np.ascontiguousarray(np.asarray(W, dtype=np.float32)).astype(bf)
    b = np.ascontiguousarray(np.asarray(b, dtype=np.float32))

    nc = _get_nc()

    x_bf = np.ascontiguousarray(x).astype(bf)
    eye_np = np.eye(128, dtype=bf)
    eyef_np = np.eye(128, dtype=np.float32)
    in_maps = []
    for c in range(NCORES):
        rows = slice(c * R, (c + 1) * R)
        in_maps.append({
            "adjT": np.ascontiguousarray(adj[rows, :].T).astype(bf),
            "x": x_bf,
            "xown": x_bf[rows, :].copy(),
            "W": W,
            "b": b,
            "eye": eye_np,
            "eyef": eyef_np,
        })

    res = run_bass_kernel_spmd(nc, in_maps, core_ids=list(range(NCORES)))
    out = np.concatenate(
        [np.asarray(res.results[c]["outT"]).T for c in range(NCORES)], axis=0)
    return np.ascontiguousarray(out, dtype=np.float32)


if __name__ == "__main__":
    rng = np.random.default_rng(0)
    x = rng.standard_normal((N, D)).astype(np.float32)
    adj = rng.random((N, N)).astype(np.float32)
    W = rng.standard_normal((D, D)).astype(np.float32) * 0.06
    b = rng.standard_normal((D,)).astype(np.float32) * 0.06
    out = kernel(x=x, adj=adj, W=W, b=b)
    print(out.shape, out.dtype)


# revision 9
# speedup vs baseline: 1.4533x; 1.1189x over previous
"""GCN layer on 8 Trainium2 NeuronCores.

Computes relu(D^-1/2 (A+I) D^-1/2 X W + b) for N=8192, d=256.

Sharding: row-shard adj over N across the 8 cores (1024 rows each); x, W, b
replicated. Each core's shard is laid out column-major (adj[rows].T) and in
bf16 (the tensor-engine compute precision) so the contraction dim (adj
columns j) lands on SBUF partitions, which the PE matmul requires.

Pipeline per core (single NEFF):
  1. Stream the 16MB shard once (HWDGE) into a persistent SBUF cache, in two
     stages: first the columns for its rows 0:512 ("A"), then rows 512:1024
     ("B"). The tensor engine reduces row sums (matmul vs ones) as tiles land.
  2. AllGather #1 ships the A-half degrees while the B-half still streams;
     AllGather #2 ships the B-half. Degrees come back through a natural
     (contiguous) DMA + PE transpose into per-partition D^-1/2 tables.
  3. U^T = ((A+I) y)^T with y = D^-1/2 x: y is scaled chunk-by-chunk
     (alternating Scalar/Vector engines) just ahead of the matmuls; the
     matmuls for row-half A start after AllGather #1, hiding AllGather #2;
     +I enters via identity-matmuls of the core's own y rows.
  4. Scale by own D^-1/2 (free-dim broadcast), apply W, bias, ReLU, and
     write the output block transposed; the host stitches the 8 blocks.
"""

import numpy as np

N = 8192
D = 256
NCORES = 8
R = N // NCORES  # rows per core = 1024
KT = N // 128  # 64 j-tiles
TS = R // 128  # 8 own-row tiles

_CACHE = {}


def _build_nc():
    import concourse.bacc as bacc
    import concourse.tile as tile
    import concourse.mybir as mybir

    f32 = mybir.dt.float32
    bf16 = mybir.dt.bfloat16
    AF = mybir.ActivationFunctionType

    nc = bacc.Bacc("TRN2", target_bir_lowering=False, debug=False,
                   num_devices=NCORES)

    adjT = nc.dram_tensor("adjT", [N, R], bf16, kind="ExternalInput")
    xin = nc.dram_tensor("x", [N, D], bf16, kind="ExternalInput")
    xown = nc.dram_tensor("xown", [R, D], bf16, kind="ExternalInput")
    Win = nc.dram_tensor("W", [D, D], bf16, kind="ExternalInput")
    bin_ = nc.dram_tensor("b", [D], f32, kind="ExternalInput")
    eyeb = nc.dram_tensor("eye", [128, 128], bf16, kind="ExternalInput")
    eyef = nc.dram_tensor("eyef", [128, 128], f32, kind="ExternalInput")
    outT = nc.dram_tensor("outT", [D, R], f32, kind="ExternalOutput")

    with tile.TileContext(nc) as tc:
        from contextlib import ExitStack

        with ExitStack() as ctx:
            pp = ctx.enter_context(tc.tile_pool(name="persist", bufs=1))
            dp = ctx.enter_context(tc.tile_pool(name="dram", bufs=1, space="DRAM"))

            # ---- persistent SBUF tensors ----
            adjTb = pp.tile([128, KT * R], bf16)   # 128KB/partition cache
            xb = pp.tile([128, KT * D], bf16)      # x, partition = j%128
            xob = pp.tile([128, TS * D], bf16)     # own x rows
            Wb = pp.tile([128, 2 * D], bf16)       # W, partition = n%128
            bsb = pp.tile([128, 2], f32)           # bias, partition = m%128
            eye_s = pp.tile([128, 128], bf16)
            eyef_s = pp.tile([128, 128], f32)
            ones_bf = pp.tile([128, 1], bf16)
            deg_s = pp.tile([1, R], f32)           # local degree (+1), A|B halves
            disl = pp.tile([1, R], f32)            # local D^-1/2
            degnA = pp.tile([32, 128], f32)        # gathered A degrees, natural
            degnB = pp.tile([32, 128], f32)
            deglnA = pp.tile([4, 128], f32)        # local A degrees, natural
            deglnB = pp.tile([4, 128], f32)
            disA = pp.tile([128, 32], f32)         # D^-1/2 for j-tiles k%8<4
            disB = pp.tile([128, 32], f32)         # D^-1/2 for j-tiles k%8>=4
            diso = pp.tile([128, TS], f32)         # own D^-1/2, partition = i%128
            disrep = pp.tile([128, R], f32)        # own D^-1/2 on free dim
            y2 = [pp.tile([128, R], bf16, name=f"y2_{i}") for i in range(2)]
            outsb = [pp.tile([128, R], f32, name=f"outsb_{i}") for i in range(2)]

            deglA_d = dp.tile([R // 2], f32)
            deglB_d = dp.tile([R // 2], f32)
            degaA_d = dp.tile([N // 2], f32)
            degaB_d = dp.tile([N // 2], f32)
            disl_d = dp.tile([R], f32)

            nc.any.memset(ones_bf[:], 1.0)

            # ---- small loads first (sync HWDGE queue) ----
            nc.sync.dma_start(
                out=xob[:, :].rearrange("p (t n) -> p t n", n=D),
                in_=xown.ap().rearrange("(t p) n -> p t n", p=128))
            nc.sync.dma_start(
                out=Wb[:, :].rearrange("p (k m) -> p k m", m=D),
                in_=Win.ap().rearrange("(k p) m -> p k m", p=128))
            nc.sync.dma_start(
                out=bsb[:, :], in_=bin_.ap().rearrange("(h p) -> p h", p=128))
            nc.sync.dma_start(out=eye_s[:, :], in_=eyeb.ap())
            nc.sync.dma_start(out=eyef_s[:, :], in_=eyef.ap())

            # ---- phase 1: stream adjT in half-column stages + row sums ----
            GC = 8  # j-tiles per DMA chunk
            adjTb3 = adjTb[:, :].rearrange("p (k i) -> p k i", i=R)
            xb3 = xb[:, :].rearrange("p (k n) -> p k n", n=D)
            xin3 = xin.ap().rearrange("(k p) n -> p k n", p=128)

            def stream_half(s):
                lo, hi = s * 512, (s + 1) * 512
                for g in range(KT // GC):
                    src = adjT.ap()[g * GC * 128:(g + 1) * GC * 128,
                                    lo:hi].rearrange("(k p) i -> p k i", p=128)
                    nc.sync.dma_start(
                        out=adjTb3[:, g * GC:(g + 1) * GC, lo:hi], in_=src)

            stream_half(0)                       # rows A of all cores
            nc.sync.dma_start(out=xb3[:, 0:KT // 2, :],
                              in_=xin3[:, 0:KT // 2, :])
            stream_half(1)                       # rows B
            nc.sync.dma_start(out=xb3[:, KT // 2:KT, :],
                              in_=xin3[:, KT // 2:KT, :])

            pdeg = ctx.enter_context(tc.tile_pool(name="psdeg", bufs=1, space="PSUM"))
            pst = ctx.enter_context(tc.tile_pool(name="pst", bufs=1, space="PSUM"))
            psuo = ctx.enter_context(tc.tile_pool(name="psuo", bufs=2, space="PSUM"))

            dps = pdeg.tile([1, 1024], f32, padded_shape=[128, 1024])
            for s in range(2):
                for k in range(KT):
                    nc.tensor.matmul(
                        dps[:, s * 512:(s + 1) * 512], ones_bf[:, :],
                        adjTb[:, k * R + s * 512:k * R + (s + 1) * 512],
                        start=(k == 0), stop=(k == KT - 1),
                        skip_group_check=True)
                # deg = rowsum + 1 (the +I term)
                nc.vector.tensor_scalar_add(
                    deg_s[:, s * 512:(s + 1) * 512],
                    dps[:, s * 512:(s + 1) * 512], 1.0)
                # ship this half's degrees
                dsts = [deglA_d, deglB_d]
                nc.scalar.dma_start(out=dsts[s][:],
                                    in_=deg_s[0:1, s * 512:(s + 1) * 512])

            nc.gpsimd.collective_compute(
                "AllGather", mybir.AluOpType.bypass,
                replica_groups=[list(range(NCORES))],
                ins=[deglA_d.opt()], outs=[degaA_d.opt()])
            nc.gpsimd.collective_compute(
                "AllGather", mybir.AluOpType.bypass,
                replica_groups=[list(range(NCORES))],
                ins=[deglB_d.opt()], outs=[degaB_d.opt()])

            # local dis for the free-dim broadcast (via DRAM round trip)
            nc.vector.reciprocal_approx_fast(disl[:, :], deg_s[:, :])
            nc.scalar.activation(disl[:, :], disl[:, :], AF.Sqrt)
            nc.scalar.dma_start(out=disl_d[:], in_=disl[0:1, :])
            nc.scalar.dma_start(
                out=disrep[:, :],
                in_=disl_d.opt().unsqueeze(0).partition_broadcast(128))

            # gathered degrees -> per-partition D^-1/2 via PE transpose
            tall = pst.tile([128, 72], f32)
            for s, (dega, degn, dis, lo) in enumerate(
                    [(degaA_d, degnA, disA, 0), (degaB_d, degnB, disB, 32)]):
                nc.scalar.dma_start(
                    out=degn[:, :],
                    in_=dega.opt().rearrange("(c f) -> c f", f=128))
                nc.tensor.transpose(tall[:, lo:lo + 32], degn[:, :],
                                    eyef_s[0:32, 0:32])
                nc.vector.reciprocal_approx_fast(dis[:, :], tall[:, lo:lo + 32])
                nc.scalar.activation(dis[:, :], dis[:, :], AF.Sqrt)
            # local degrees -> own D^-1/2 table (for the +I rows)
            for s, (degl, degln, lo) in enumerate(
                    [(deglA_d, deglnA, 64), (deglB_d, deglnB, 68)]):
                nc.scalar.dma_start(
                    out=degln[:, :],
                    in_=degl.opt().rearrange("(c f) -> c f", f=128))
                nc.tensor.transpose(tall[:, lo:lo + 4], degln[:, :],
                                    eyef_s[0:4, 0:4])
                nc.vector.reciprocal_approx_fast(
                    diso[:, s * 4:(s + 1) * 4], tall[:, lo:lo + 4])
                nc.scalar.activation(diso[:, s * 4:(s + 1) * 4],
                                     diso[:, s * 4:(s + 1) * 4], AF.Sqrt)

            # ---- phase 3: y = dis*x and U^T = ((A+I) y)^T ----
            u = [psuo.tile([128, R], f32, name=f"u_{i}", tag="uo") for i in range(2)]

            def dis_col(k):
                c, t = divmod(k, 8)
                if t < 4:
                    return disA[:, 4 * c + t:4 * c + t + 1]
                return disB[:, 4 * c + t - 4:4 * c + t - 3]

            def scale_y(k):
                chunk = xb[:, k * D:(k + 1) * D]
                if k % 2 == 0:
                    nc.scalar.activation(chunk, chunk, AF.Copy,
                                         scale=dis_col(k))
                else:
                    nc.vector.tensor_scalar_mul(chunk, chunk, dis_col(k))

            ksA = [k for k in range(KT) if k % 8 < 4]
            ksB = [k for k in range(KT) if k % 8 >= 4]
            # rows-A matmuls: unlocked by AllGather #1, hide AllGather #2
            for k in ksA:
                scale_y(k)
                for h in range(2):
                    nc.tensor.matmul(
                        u[h][:, 0:512],
                        xb[:, k * D + h * 128:k * D + (h + 1) * 128],
                        adjTb[:, k * R:k * R + 512],
                        start=(k == ksA[0]), stop=False,
                        skip_group_check=True)
            for k in ksA:
                for h in range(2):
                    nc.tensor.matmul(
                        u[h][:, 512:1024],
                        xb[:, k * D + h * 128:k * D + (h + 1) * 128],
                        adjTb[:, k * R + 512:k * R + 1024],
                        start=(k == ksA[0]), stop=False,
                        skip_group_check=True)
            # +I: U^T[n, own block t] += y_own[t]^T
            for t in range(TS):
                chunk = xob[:, t * D:(t + 1) * D]
                nc.scalar.activation(chunk, chunk, AF.Copy,
                                     scale=diso[:, t:t + 1])
                for h in range(2):
                    nc.tensor.matmul(
                        u[h][:, t * 128:(t + 1) * 128],
                        xob[:, t * D + h * 128:t * D + (h + 1) * 128],
                        eye_s[:, :],
                        start=False, stop=False, skip_group_check=True)
            # rows-B matmuls: unlocked by AllGather #2
            for k in ksB:
                scale_y(k)
                for h in range(2):
                    for s in range(2):
                        nc.tensor.matmul(
                            u[h][:, s * 512:(s + 1) * 512],
                            xb[:, k * D + h * 128:k * D + (h + 1) * 128],
                            adjTb[:, k * R + s * 512:k * R + (s + 1) * 512],
                            start=False, stop=(k == ksB[-1]),
                            skip_group_check=True)

            # ---- phase 4: scale columns by own dis, cast to bf16 ----
            for h in range(2):
                nc.vector.tensor_mul(y2[h][:, :], u[h][:, :], disrep[:, :])

            # ---- phase 5: out^T = W^T @ (scaled U^T) ----
            o = [psuo.tile([128, R], f32, name=f"o_{i}", tag="uo") for i in range(2)]
            for mh in range(2):
                for nk in range(2):
                    for s in range(2):
                        nc.tensor.matmul(
                            o[mh][:, s * 512:(s + 1) * 512],
                            Wb[:, nk * D + mh * 128:nk * D + (mh + 1) * 128],
                            y2[nk][:, s * 512:(s + 1) * 512],
                            start=(nk == 0), stop=(nk == 1),
                            skip_group_check=True)

            # ---- phase 6: relu(out^T + b), write transposed output ----
            for mh in range(2):
                nc.scalar.activation(
                    outsb[mh][:, :], o[mh][:, :], AF.Relu,
                    bias=bsb[:, mh:mh + 1], scale=1.0)
                nc.sync.dma_start(
                    out=outT.ap()[mh * 128:(mh + 1) * 128, :],
                    in_=outsb[mh][:, :])

    nc.compile()
    return nc


def _get_nc():
    if "nc" not in _CACHE:
        _CACHE["nc"] = _build_nc()
    return _CACHE["nc"]


def kernel(x, adj, W, b):
    import ml_dtypes
    from concourse.bass_utils import run_bass_kernel_spmd

    bf = ml_dtypes.bfloat16
    x = np.asarray(x, dtype=np.float32)
    adj = np.asarray(adj, dtype=np.float32)
    W = np.ascontiguousarray(np.asarray(W, dtype=np.float32)).astype(bf)
    b = np.ascontiguousarray(np.asarray(b, dtype=np.float32))

    nc = _get_nc()

    x_bf = np.ascontiguousarray(x).astype(bf)
    eye_np = np.eye(128, dtype=bf)
    eyef_np = np.eye(128, dtype=np.float32)
    in_maps = []
    for c in range(NCORES):
        rows = slice(c * R, (c + 1) * R)
        in_maps.append({
            "adjT": np.ascontiguousarray(adj[rows, :].T).astype(bf),
            "x": x_bf,
            "xown": x_bf[rows, :].copy(),
            "W": W,
            "b": b,
            "eye": eye_np,
            "eyef": eyef_np,
        })

    res = run_bass_kernel_spmd(nc, in_maps, core_ids=list(range(NCORES)))
    out = np.concatenate(
        [np.asarray(res.results[c]["outT"]).T for c in range(NCORES)], axis=0)
    return np.ascontiguousarray(out, dtype=np.float32)


if __name__ == "__main__":
    rng = np.random.default_rng(0)
    x = rng.standard_normal((N, D)).astype(np.float32)
    adj = rng.random((N, N)).astype(np.float32)
    W = rng.standard_normal((D, D)).astype(np.float32) * 0.06
    b = rng.standard_normal((D,)).astype(np.float32) * 0.06
    out = kernel(x=x, adj=adj, W=W, b=b)
    print(out.shape, out.dtype)
